# revision 33
# baseline (speedup 1.0000x reference)
"""Bipolar LIF neuron forward pass on 8 Trainium2 NeuronCores.

Reference semantics (all fp32, per element over [B, N, F], recurrence over T):
    V_t   = alpha * q_{t-1} + x_t           (q_{-1} = 0)
    pos_t = (V_t >= 1.0)                    -> out[..., :F]
    neg_t = (V_t <= -1.0)                   -> out[..., F:]
    q_t   = V_t - (pos_t + neg_t)           (both spike types subtract 1)

Sharding: data-parallel over B (8 batches -> 8 cores, no communication).

Device-side encoding: the chain carries the PRE-reset potential V_t and two
custom DVE ops do all the work per timestep:
  VSTEP: V_t = (V_{t-1} - ((V_{t-1} > c) + (V_{t-1} < -c))) * alpha + x_t
         (c = pred(1.0), so "> c" == ">= 1.0"; one 2-input fused op)
  SPK:   s_t = (V_t > c) - (V_t < -c)  in {-1, 0, +1}, written as int8
         (packed spike plane; fp32 {-1,0,1} -> int8 convert is exact)
The host decodes pos = (s == 1), neg = (s == -1). Storing one int8 plane
instead of pos/neg uint8 planes halves store traffic (24 MiB -> 20 MiB
per-core round trip, the HBM roofline for this problem).
"""

import os
import sys

for _p in ("/opt/trn_rl_repo",):
    if _p not in sys.path and os.path.isdir(_p):
        sys.path.insert(0, _p)

from contextlib import ExitStack

import numpy as np

import concourse.bass as bass  # noqa: F401  (AP types come through tile/bacc)
import concourse.tile as tile
from concourse import bacc, mybir
from concourse.bass_utils import run_bass_kernel_spmd

B, T, N, F = 8, 32, 1024, 128
P = 128          # SBUF partitions
W = N * F // P   # free elements per partition per timestep (1024)
ALPHA = float(np.float32(np.exp(np.float32(-1.0 / 20.0))))
# Strict threshold shift: V >= 1.0f  <=>  V > pred(1.0f).
CPRED = float(np.nextafter(np.float32(1.0), np.float32(0.0)))

_NC_CACHE = {}


def _register_ops():
    """Register the two custom DVE ops (idempotent).

    VSTEP_ANT: out = (Src0 - ((Src0 > C1) - (Src0 < C2))) * C0 + Src1
    SPK_ANT:   out = (Src0 > C1) - (Src0 < C2)
    With s0=ALPHA, s1=CPRED, imm2=-CPRED these are the fused LIF chain step
    and the packed bipolar spike. Both bit-exact vs the fp32 reference
    (mult and add lower to separate ALU blocks -> same rounding as jax).
    """
    import copy

    import concourse.dve_ops as dve_ops
    from concourse.dve_ops import DveOp
    from concourse.dve_spec import C0, C1, Spec, Src0, Src1, Zero, lower
    from concourse.dve_uop import (
        AluInp,
        AluOp,
        DelayInp,
        DveOpSpec,
        ENABLE,
        InpSel,
        OutPath,
        OutSel,
        Trigger,
        UopConfig,
    )
    from concourse.dve_ops import get_dve_sub_opcode

    have = {o.name: o for o in dve_ops.OPS}

    def _spk_2x_steady(second_src, two_port):
        """Hand-written 2-element/cycle steady uOp for SPK, following the
        stock tensor_scalar perf-mode programs (dve_bin_gen3 slots 17/18):
        2X_2PORT reads element B via SRC_1 (second read port) and writes it
        via WR1_LO; 2X_1PORT (never engaged for fp32, table slot must still
        be valid) uses SRC_0_HI / WR0_HI. Element A: chain1 -> b0 gt, b1 lt,
        b2 s=gt-lt -> parked on chain3. Element B: chain2 -> b3 gt, b4 lt,
        b5 s -> bypassed to b7's ALU_OUT. CURR_SWAP_OUT holds -C1 from the
        shared latch-init uOp."""
        u = UopConfig()
        u.enable_input(InpSel.CONST_1, 1)  # chain0 = C1
        u.enable_input(InpSel.SRC_0, 2)    # chain1 = vA
        u.enable_input(second_src, 3)      # chain2 = vB
        u.require_inp0 = ENABLE
        u.require_inp1 = ENABLE if two_port else 0
        u.trigger = (Trigger.SRC_TENSOR_DONE, Trigger.NONE, Trigger.NONE)
        dp = u.datapath_config
        dp[0].enable_alu(AluOp.IS_LT, AluInp.PREV_DELAY_0, AluInp.PREV_DELAY_1)
        dp[0].pass_through_delay(0, 1, 2)
        dp[1].enable_alu(AluOp.IS_LT, AluInp.PREV_DELAY_1, AluInp.CURR_SWAP_OUT)
        dp[1].pass_through_delay(0, 2)
        dp[1].enable_delay_from_src(DelayInp.PREV_ALU_OUT, 1)  # chain1 <- gtA
        dp[2].enable_alu(AluOp.SUBTRACT, AluInp.PREV_DELAY_1, AluInp.PREV_ALU_OUT)
        dp[2].pass_through_delay(0, 2)
        dp[3].enable_alu(AluOp.IS_LT, AluInp.PREV_DELAY_0, AluInp.PREV_DELAY_2)
        dp[3].pass_through_delay(2)
        dp[3].enable_delay_from_src(DelayInp.PREV_ALU_OUT, 3)  # chain3 <- sA
        dp[4].enable_alu(AluOp.IS_LT, AluInp.PREV_DELAY_2, AluInp.CURR_SWAP_OUT)
        dp[4].pass_through_delay(3)
        dp[4].enable_delay_from_src(DelayInp.PREV_ALU_OUT, 2)  # chain2 <- gtB
        dp[5].enable_alu(AluOp.SUBTRACT, AluInp.PREV_DELAY_2, AluInp.PREV_ALU_OUT)
        dp[5].pass_through_delay(3)
        dp[6].pass_through_alu()
        dp[6].pass_through_delay(3)
        dp[7].pass_through_alu()
        dp[7].pass_through_delay(3)
        u.enable_output(OutSel.DELAY_3, OutPath.WR0_LO)          # sA
        u.enable_output(
            OutSel.ALU_OUT, OutPath.WR1_LO if two_port else OutPath.WR0_HI
        )                                                        # sB
        return u

    class AntDveOp(DveOp):
        """DveOp whose uops_sha is computed at registration time (the spec
        is defined in this file, so there is no golden to pin against).
        `perf2x=True` attaches the hand-written 2x perf-mode programs."""

        perf2x: bool = False

        def compile(self, ver):
            key = (self.name, ver)
            cache = dve_ops._COMPILE_CACHE
            if (r := cache.get(key)) is not None:
                return r
            base = lower(self.spec, ver=ver)
            kw = {}
            if self.perf2x and ver == "v3":
                # The swap (latch) flop is PER BLOCK: the base latch-init
                # writes -C1 only into the block the 1x body reads it at
                # (b1). Element B's IS_LT reads CURR_SWAP_OUT at b4, so the
                # 2x init must latch there too (the value reaches b4 via the
                # init's ALU bypass chain).
                init = copy.deepcopy(base[0])
                init.datapath_config[4].swap_enable = ENABLE
                kw = dict(
                    uops_2x=[copy.deepcopy(init), _spk_2x_steady(InpSel.SRC_0_HI, False)],
                    uops_2x_2p=[copy.deepcopy(init), _spk_2x_steady(InpSel.SRC_1, True)],
                    perf_max=2,
                )
            result = DveOpSpec(
                name=self.name,
                opcode=get_dve_sub_opcode(self.name),
                uops=base,
                rd1_en=dve_ops.has_src1(self.spec),
                **kw,
            )
            cache[key] = result
            return result

    def reg(name, spec, perf2x=False):
        if name in have:
            return have[name]
        op = AntDveOp(name, spec, subdim=False, uops_sha={})
        object.__setattr__(op, "perf2x", perf2x)
        dve_ops.OPS.append(op)
        dve_ops.CUSTOM_DVE_SPECS[name] = op.spec
        dve_ops._SUB_OPCODE_FOR_NAME[name] = (
            dve_ops._CUSTOM_DVE_ROW_BASE + len(dve_ops.OPS) - 1
        )
        return op

    # -C1 via the stream-invariant (Zero - C1): hoisted to latch-init, so
    # both ops need only the two TTSS/STT scalar slots (s0, s1) and the
    # 2-input VSTEP can use the STT struct (elementwise 3-D src1).
    _nC1 = Zero - C1
    _spk = (Src0 > C1) - (Src0 < _nC1)
    _v = (Src0 - ((Src0 > C1) + (Src0 < (Zero - C1)))) * C0 + Src1
    # Two timesteps packed into one int8: s_t + 16*s_{t+1} (both in {-1,0,1},
    # all intermediate fp32 values are small ints -> exact).
    _spk2 = ((Src0 > C1) - (Src0 < (Zero - C1))) + (
        ((Src1 > C1) - (Src1 < (Zero - C1))) * C0
    )

    def _spk_np(v, s1):
        return (v > np.float32(s1)).astype(np.float32) - (
            v < -np.float32(s1)
        ).astype(np.float32)

    def _spk_ref(in0, in1, s0, s1, imm2):
        return _spk_np(in0.astype(np.float32), s1)

    def _spk2_ref(in0, in1, s0, s1, imm2):
        return _spk_np(in0.astype(np.float32), s1) + _spk_np(
            in1.astype(np.float32), s1
        ) * np.float32(s0)

    def _vstep_ref(in0, in1, s0, s1, imm2):
        v = in0.astype(np.float32)
        r = (v > np.float32(s1)).astype(np.float32) + (
            v < -np.float32(s1)
        ).astype(np.float32)
        return ((v - r) * np.float32(s0)).astype(np.float32) + in1

    vstep = reg("LIF_VSTEP_ANT", Spec(body=_v, reference=_vstep_ref))
    spk = reg("LIF_SPK_ANT", Spec(body=_spk, reference=_spk_ref), perf2x=True)
    spk2 = reg("LIF_SPK2_ANT", Spec(body=_spk2, reference=_spk2_ref))
    return vstep, spk, spk2


NPAIR = (T - 2) // 2  # timestep pairs (2k, 2k+1), k < NPAIR; 30, 31 are singles


def _build_program():
    f32 = mybir.dt.float32
    i8 = mybir.dt.int8
    vstep_op, spk_op, spk2_op = _register_ops()

    nc = bacc.Bacc(
        "TRN2",
        target_bir_lowering=False,
        debug=False,
        enable_asserts=False,
    )
    J = 8  # W folded as [J, F] so the 2-input ops get a 3-D (STT) src1
    x_d = nc.dram_tensor("x", [T, P, J, F], f32, kind="ExternalInput").ap()
    # Rows 0..NPAIR-1: packed pairs s_2k + 16*s_2k+1; rows NPAIR, NPAIR+1:
    # single planes for t = 30, 31 (kept single so the tail stays short).
    y_d = nc.dram_tensor("y", [NPAIR + 2, P, J, F], i8, kind="ExternalOutput").ap()

    HS = ((0, J // 2), (J // 2, J))

    with tile.TileContext(nc) as tc, ExitStack() as ctx:
        xpool = ctx.enter_context(tc.tile_pool(name="xin", bufs=10))
        vpool = ctx.enter_context(tc.tile_pool(name="vch", bufs=5))
        spool = ctx.enter_context(tc.tile_pool(name="spk", bufs=8))

        def spk(sp, vt, h0, h1):
            bi = nc.vector._custom_dve(
                spk_op, out=sp[:, h0:h1, :], in0=vt[:, h0:h1, :],
                s0=0.0, s1=CPRED,
            )
            # byte-36[7:6]: allow the engine up to the 2X_2PORT perf slot
            # (single-source, SBUF, even major dim -> 2 elem/cycle); the
            # table rows for slots +1/+2 are written by AntDveOp.compile.
            bi.ins.perf_max = 2

        def load(t, xt, eng):
            # The chain is load-gated until loads pull ahead (~step 6):
            # split early loads so each VSTEP half can start one
            # half-transfer earlier. x_0 goes on the SP queue (fastest DGE
            # path, no stores queued yet); the rest on the ACT queue so
            # store-DMA sem waits on SP can't head-of-line-block prefetch.
            if t <= 5:
                for h0, h1 in HS:
                    eng.dma_start(out=xt[:, h0:h1, :], in_=x_d[t][:, h0:h1])
            else:
                eng.dma_start(out=xt[:], in_=x_d[t])

        x_cur = xpool.tile([P, J, F], f32, name="xt")
        load(0, x_cur, nc.sync)
        x_next = xpool.tile([P, J, F], f32, name="xt")
        load(1, x_next, nc.scalar)
        # V_0 = alpha*0 + x_0 = x_0: use the loaded tile directly.
        vs = {0: x_cur}
        pend_store = None  # pair store delayed one pair: ramp loads go first
        for t in range(1, T):
            xt = x_next
            if t + 1 < T:
                x_next = xpool.tile([P, J, F], f32, name="xt")
                load(t + 1, x_next, nc.scalar)
            if t == T - 1:
                # SPK_30 + its store issue BEFORE VSTEP_31 (V_30's ack has
                # SPK2_14's 1127ns in between), so after the chain's last op
                # only SPK_31's halves remain on the critical tail.
                sp30 = spool.tile([P, J, F], i8, name="sp")
                spk(sp30, vs[t - 1], 0, J)
                nc.scalar.dma_start(out=y_d[NPAIR], in_=sp30[:])
            vt = vpool.tile([P, J, F], f32, name="vt")
            for h0, h1 in HS if (t <= 5 or t == T - 1) else ((0, J),):
                nc.vector._custom_dve(
                    vstep_op, out=vt[:, h0:h1, :], in0=vs[t - 1][:, h0:h1, :],
                    in1=xt[:, h0:h1, :], s0=ALPHA, s1=CPRED,
                )
            vs[t] = vt
            # Packed pair (t-2, t-1) sits between VSTEP_t and VSTEP_{t+1}:
            # it reads V tiles acked at least one op ago, filling the DVE
            # bubble where VSTEP_{t+1} would stall on V_t's SBUF write-ack.
            if t % 2 == 0 and t >= 2:
                k = (t - 2) // 2
                sp2 = spool.tile([P, J, F], i8, name="sp")
                nc.vector._custom_dve(
                    spk2_op, out=sp2[:], in0=vs[t - 2][:], in1=vs[t - 1][:],
                    s0=16.0, s1=CPRED,
                )
                if pend_store is not None:
                    nc.sync.dma_start(out=y_d[pend_store[0]], in_=pend_store[1][:])
                pend_store = (k, sp2)
                if k == NPAIR - 1:
                    nc.sync.dma_start(out=y_d[k], in_=sp2[:])
                    pend_store = None
        # Tail: t = 31 in halves so the first half-store overlaps the
        # second half's SPK.
        sp31 = spool.tile([P, J, F], i8, name="sp")
        for (h0, h1), eng in zip(HS, (nc.scalar, nc.sync)):
            spk(sp31, vs[T - 1], h0, h1)
            # The two final half-stores issue from different queues (ACT's
            # load stream is drained by then) so their ~650ns SEQ+HWDGE
            # issue paths overlap instead of serializing on SP.
            eng.dma_start(out=y_d[NPAIR + 1][:, h0:h1], in_=sp31[:, h0:h1])

    nc.compile()
    return nc


def get_program():
    if "nc" not in _NC_CACHE:
        _NC_CACHE["nc"] = _build_program()
    return _NC_CACHE["nc"]


def kernel(input_current: np.ndarray, _return_bench=False, **_bench_kwargs):
    assert input_current.shape == (B, T, N, F), input_current.shape
    xs = np.ascontiguousarray(input_current, dtype=np.float32).reshape(
        B, T, P, W // F, F
    )
    in_maps = [{"x": xs[b]} for b in range(B)]
    nc = get_program()
    res = run_bass_kernel_spmd(nc, in_maps, core_ids=list(range(B)), **_bench_kwargs)
    # Device stores spikes as int8: rows 0..NPAIR-1 hold packed pairs
    # s_2k + 16*s_2k+1 (each in {-1,0,1}), the last two rows hold single
    # planes for t = T-2, T-1. Expand to [pos | neg] fp32 planes on host.
    out = np.empty((B, T, N, 2 * F), dtype=np.float32)
    s = np.empty((T, N, F), dtype=np.int8)
    for b in range(B):
        y = res.results[b]["y"].reshape(NPAIR + 2, N, F)
        pairs = y[:NPAIR].astype(np.int16)
        s_odd = (pairs + 24) // 16 - 1          # {-17..17} -> s_{2k+1}
        s[1 : 2 * NPAIR : 2] = s_odd
        s[0 : 2 * NPAIR : 2] = (pairs - 16 * s_odd).astype(np.int8)
        s[T - 2] = y[NPAIR]
        s[T - 1] = y[NPAIR + 1]
        out[b, :, :, :F] = (s == 1)
        out[b, :, :, F:] = (s == -1)
    if _return_bench:
        return out, res
    return out


if __name__ == "__main__":
    x = np.random.randn(B, T, N, F).astype(np.float32)
    y = kernel(x)
    print("kernel output:", y.shape, y.dtype, "mean", y.mean())
